# revision 27
# baseline (speedup 1.0000x reference)
"""BinNorm (sum-of-sigmoids row normalization via root-find) for Trainium2.

Math: for each row x of shape [256], find nu s.t. sum(sigmoid(x + nu)) == 64,
then output sigmoid(x + nu).  The reference's lattice bisection quantizes nu
to a bracket of width ~6.8e-5; any nu within ~1e-3 of the true root keeps the
output within ~2.5e-4 of the reference (sigmoid slope <= 1/4).

Kernel algorithm per row:
  1. row moments via bn_stats  ->  nu0 = c0 + c1*m + (c2 + c3*v)*v
     (least-squares fit of the true root over N(0,1) rows; max err ~0.038)
  2. one Newton step with a fitted reciprocal slope:
        f0  = sum sigmoid(x + nu0)
        nu1 = nu0 - (f0 - 64) * (a0 + a1*v)
     (max residual ~8e-4 in nu -> ~2e-4 in the output)
  3. output pass  sigmoid(x + nu1)

Step 3 is applied as a first-order in-place fix instead of a second sigmoid
pass: with out0 = sigmoid(x + nu0) and delta = nu1 - nu0,
    out ~= out0 + delta*out0*(1-out0) = (out0*(-delta) + (1+delta)) * out0,
one AFFINE_MUL_REDUCE custom-DVE instruction per tile (the quadratic
remainder is <= delta^2/2 * max|sigmoid''| ~ 7e-5).  This removes the second
ACT pass, every pre-add, and every DVE row-sum from the steady state:
  ACT : sigmoid(x, bias=nu0, accum_out=f0) per tile, writing out0 straight
        into the store-DMA block (585ns incl. the accumulator read)
  DVE : bn_stats+bn_aggr per tile (394ns) + the affine fix (327ns)
  Pool: init polynomials + delta/c1 smalls only
  DMA : 728ns/tile in+out -- the 11.65us roofline this schedule chases
The 16 row-tiles per core stream through a software pipeline of "units"
(1-2 tiles): stats batches are emitted a few tiles ahead (BK_SLA), the fix
stage lags one unit (BK_LAG), stores fire per 2-tile block.  Unit fix forms
d (DVE affine) / a (ACT sigmoid bias=nu2) / p (2-op Pool) are knob-tunable;
the all-'d' schedule with a pair-first load layout (2,1,1,2,2,2,2,4)
measures best: 19310ns modeled, ACT and DVE both ~11us busy.

Sharding: pure data parallel over rows, 8 cores x 2048 rows.
"""

import os as _os
import numpy as np

_CORES = 8
_B, _D = 16384, 256
_BC = _B // _CORES          # rows per core
_P = 128                    # partitions
_T = _BC // _P              # 16 row-tiles per core

# nu0 = C[0] + C[1]*m + (C[2] + C[3]*v)*v   (m=row mean, v=row var)
_C = (-1.1054261909417549, -1.0002364201254597,
      -0.2275464721729869, 0.0177988072676918)
# 1/f'(root) ~= G[0] + G[1]*v
_G = (0.02112157406163301, 0.0033098367152893152)
_KF = 64.0

# ---- schedule knobs ------------------------------------------------------
# units: <eval><width><fix>; eval a (ACT accum f0) | c (per-tile sigmoid +
# DVE row-sum); fix d (DVE in-place affine) | a (ACT sigmoid bias=nu2) |
# w (pre-add nu2 + wide ACT sigmoid)
_UNITS = _os.environ.get(
    "BK_UNITS", "a1d,a1d,a2d,a2d,a2d,a2d,a2d,a2d,a2d")
_IN_BLOCKS = tuple(int(v) for v in _os.environ.get(
    "BK_IN_BLOCKS", "2,1,1,2,2,2,2,4").split(","))
_OUT_BLOCKS = tuple(int(v) for v in _os.environ.get(
    "BK_OUT_BLOCKS", "2,2,2,2,2,2,2,2").split(","))
# poly batches: <width-in-tiles><engine d|p>
_POLY = _os.environ.get("BK_POLY", "1d,1d,2p,2p,2p,2p,2p,2p,2p")
_DELTA_ENG = _os.environ.get("BK_DELTA_ENG", "ppppppppp")   # per unit
_EVP = _os.environ.get("BK_EVP", "ppppppppp")               # per unit (b)
_OUP = _os.environ.get("BK_OUP", "ppppppppp")               # per unit (w)
_SLA = int(_os.environ.get("BK_SLA", "5"))    # stats lookahead (tiles)
# per-round emission order: p=polys, s=stats-prefetch, 1=stage1, 2=stage2
_EMIT = _os.environ.get("BK_EMIT", "ps12")
_LAG = int(_os.environ.get("BK_LAG", "1"))    # stage2 lag (units)
_NEWTON = _os.environ.get("BK_NEWTON", "1") == "1"

_cache: dict = {}


def _build_nc():
    from contextlib import ExitStack
    import concourse.bacc as bacc
    import concourse.bass as bass
    import concourse.mybir as mybir
    import concourse.tile as tile

    f32 = mybir.dt.float32
    SIG = mybir.ActivationFunctionType.Sigmoid
    A = mybir.AluOpType
    AX = mybir.AxisListType

    units = []
    for tok in _UNITS.split(","):
        units.append((tok[0], int(tok[1]), tok[2]))  # (type, width, out)
    NU = len(units)
    assert sum(w for _, w, _ in units) == _T
    unit_t0 = []
    _acc = 0
    for (_ty, w, _om) in units:
        unit_t0.append(_acc)
        _acc += w

    pbatches = []
    _acc = 0
    for tok in _POLY.split(","):
        pbatches.append((_acc, int(tok[:-1]), tok[-1]))
        _acc += int(tok[:-1])
    assert _acc == _T
    assert sum(_IN_BLOCKS) == _T and sum(_OUT_BLOCKS) == _T

    nc = bacc.Bacc(
        "TRN2",
        target_bir_lowering=False,
        debug=False,
        enable_asserts=False,
        num_devices=_CORES,
    )
    x = nc.dram_tensor("x", [_BC, _D], f32, kind="ExternalInput").ap()
    y = nc.dram_tensor("y", [_BC, _D], f32, kind="ExternalOutput").ap()

    def bcast(ap2d, g, d=_D):
        """[P, g] AP -> [P, g, d] stride-0 broadcast view."""
        return bass.AP(ap2d.tensor, ap2d.offset,
                       [ap2d.ap[0], [ap2d.ap[1][0], g], [0, d]])

    def widen(col, g):
        """[P, 1] column AP -> contiguous [P, g] AP."""
        return bass.AP(col.tensor, col.offset,
                       [col.ap[0], [col.ap[1][0], g]])

    needs_pp = any(fx in ("p", "w") for _t, _w, fx in units) or not _NEWTON

    with tile.TileContext(nc) as tc, ExitStack() as ctx:
        xp = ctx.enter_context(tc.tile_pool(name="xp", bufs=1))
        pp = (ctx.enter_context(tc.tile_pool(name="pp", bufs=1))
              if needs_pp else None)
        op = ctx.enter_context(tc.tile_pool(name="op", bufs=1))
        st = ctx.enter_context(tc.tile_pool(name="st", bufs=1))

        eng = {"d": nc.vector, "p": nc.gpsimd}

        # warmup: trigger the sigmoid table load before any data arrives
        wz = st.tile([_P, 1], f32, tag="wz", name="wz")
        nc.vector.memset(wz[:], 0.0)
        wo = st.tile([_P, 1], f32, tag="wo", name="wo")
        nc.scalar.activation(wo[:], wz[:], SIG, bias=wz[:])

        # ---- blocked loads ----
        xt = [None] * _T            # [P, D] column views per tile
        xcol = [None] * _T          # (block tile, col) per tile
        t = 0
        for b, w in enumerate(_IN_BLOCKS):
            blk = xp.tile([_P, w * _D], f32, tag=f"xb{b}", name=f"xb{b}")
            src = x[t * _P:(t + w) * _P, :].rearrange("(t p) d -> p t d", p=_P)
            nc.sync.dma_start(blk[:].rearrange("p (t d) -> p t d", d=_D), src)
            for j in range(w):
                xt[t + j] = blk[:, (j * _D):(j + 1) * _D]
                xcol[t + j] = (blk, j)
            t += w

        def xwide(t0, w):
            """contiguous [P, w, D] view over x tiles t0..t0+w-1"""
            blk, c0 = xcol[t0]
            blkN, cN = xcol[t0 + w - 1]
            assert blk is blkN and cN == c0 + w - 1, (t0, w)
            return blk[:, c0 * _D:(c0 + w) * _D].rearrange(
                "p (g d) -> p g d", d=_D)

        # ---- store blocks ----
        oblk = []                   # [blk, t0, w]
        ocol = [None] * _T          # (store blk, col) per tile
        t = 0
        for b, w in enumerate(_OUT_BLOCKS):
            blk = op.tile([_P, w * _D], f32, tag=f"ob{b}", name=f"ob{b}")
            oblk.append([blk, t, w])
            for j in range(w):
                ocol[t + j] = (blk, j)
            t += w

        outdone = [False] * _T

        def emit_ready_stores():
            for ent in list(oblk):
                blk, t0, w = ent
                if all(outdone[t] for t in range(t0, t0 + w)):
                    oblk.remove(ent)
                    dst = y[t0 * _P:(t0 + w) * _P, :].rearrange(
                        "(t p) d -> p t d", p=_P)
                    nc.sync.dma_start(dst,
                                      blk[:].rearrange("p (t d) -> p t d",
                                                       d=_D))

        # ---- stats + polys per batch ----
        nu0_col = [None] * _T
        gg_col = [None] * _T

        agg_b = [None] * len(pbatches)

        def emit_stats(bi):
            t0, bw, e = pbatches[bi]
            if e == "q":
                # raw bn6 halves; combined on Pool in emit_polys (no bn_aggr)
                bn6b = st.tile([_P, 6 * bw], f32, tag=f"b6b{bi}",
                               name=f"b6b{bi}")
                for j in range(bw):
                    nc.vector.bn_stats(bn6b[:, 6 * j:6 * (j + 1)], xt[t0 + j])
                agg_b[bi] = bn6b
                return
            agg = st.tile([_P, 2 * bw], f32, tag=f"agg{bi}", name=f"agg{bi}")
            aggv = agg[:].rearrange("p (c g) -> p c g", g=bw)
            agg_b[bi] = aggv
            for j in range(bw):
                bn6 = st.tile([_P, 6], f32, tag=f"bn6_{bi}_{j}",
                              name=f"bn6_{bi}_{j}")
                nc.vector.bn_stats(bn6[:], xt[t0 + j])
                nc.vector.bn_aggr(aggv[:, :, j], bn6[:])

        def emit_polys(bi):
            t0, bw, e = pbatches[bi]

            def bt(tag):
                return st.tile([_P, bw], f32, tag=tag, name=tag)

            if e == "q":
                # combine even/odd halves on Pool:
                #   m = (mu_e + mu_o)/2
                #   v = (M2e + M2o + 64*(mu_e - mu_o)^2) / 256 = R/256
                # with the /2 and /256 folded into the coefficients.
                pe = nc.gpsimd
                b6 = agg_b[bi][:].rearrange("p (g c) -> p g c", c=6)
                mu_e, m2e = b6[:, :, 1], b6[:, :, 2]
                mu_o, m2o = b6[:, :, 4], b6[:, :, 5]
                sm = bt(f"sm{bi}")
                pe.tensor_tensor(sm[:], mu_e, mu_o, A.add)
                dd = bt(f"dd{bi}")
                pe.tensor_tensor(dd[:], mu_e, mu_o, A.subtract)
                d8 = bt(f"d8{bi}")
                pe.tensor_scalar(d8[:], dd[:], 8.0, 0.0, A.mult, A.add)
                qq = bt(f"qq{bi}")
                pe.tensor_tensor(qq[:], d8[:], d8[:], A.mult)
                mm = bt(f"mm{bi}")
                pe.tensor_tensor(mm[:], m2e, m2o, A.add)
                rr = bt(f"rr{bi}")
                pe.tensor_tensor(rr[:], qq[:], mm[:], A.add)
                tv = bt(f"tv{bi}")
                pe.tensor_scalar(tv[:], rr[:], _C[3] / 65536.0,
                                 _C[2] / 256.0, A.mult, A.add)
                tu = bt(f"tu{bi}")
                pe.tensor_tensor(tu[:], tv[:], rr[:], A.mult)
                tw = bt(f"tw{bi}")
                pe.tensor_scalar(tw[:], sm[:], _C[1] / 2.0, _C[0],
                                 A.mult, A.add)
                nu0 = bt(f"nu0_{bi}")
                pe.tensor_tensor(nu0[:], tu[:], tw[:], A.add)
                gg = bt(f"gg{bi}")
                pe.tensor_scalar(gg[:], rr[:], _G[1] / 256.0, _G[0],
                                 A.mult, A.add)
                for j in range(bw):
                    nu0_col[t0 + j] = nu0[:, j:j + 1]
                    gg_col[t0 + j] = gg[:, j:j + 1]
                return

            pe = eng[e]
            aggv = agg_b[bi]
            m = aggv[:, 0, :]
            v = aggv[:, 1, :]
            tv = bt(f"tv{bi}")
            pe.tensor_scalar(tv[:], v, _C[3], _C[2], A.mult, A.add)
            tu = bt(f"tu{bi}")
            pe.tensor_tensor(tu[:], tv[:], v, A.mult)
            tw = bt(f"tw{bi}")
            pe.tensor_scalar(tw[:], m, _C[1], _C[0], A.mult, A.add)
            nu0 = bt(f"nu0_{bi}")
            pe.tensor_tensor(nu0[:], tu[:], tw[:], A.add)
            gg = bt(f"gg{bi}")
            pe.tensor_scalar(gg[:], v, _G[1], _G[0], A.mult, A.add)
            for j in range(bw):
                nu0_col[t0 + j] = nu0[:, j:j + 1]
                gg_col[t0 + j] = gg[:, j:j + 1]

        # ---- per-unit compute ----
        # stage1: out0 = sigmoid(x + nu0) written into the store block,
        #         f0 per tile (ACT accum or DVE row-sum), then
        #         dlp = (f0-K)*gg  (= -delta) and c1 = 1 - dlp (= 1+delta).
        # stage2 fix forms:
        #   d: out = (out0*dlp_neg... implemented as (out0*(-dlp)+c1)*out0
        #      via affine_mul_reduce, in place on the store block
        #   a: out = sigmoid(x + nu2) per tile (overwrite), nu2 = nu0 - dlp
        #   w: pre-add nu2 then one wide sigmoid (overwrite)
        dlp_u = [None] * NU
        c1_u = [None] * NU
        nu2_u = [None] * NU

        def stage1(u):
            ty, w, fx = units[u]
            t0 = unit_t0[u]
            if not _NEWTON:
                for j in range(w):
                    ob, oc = ocol[t0 + j]
                    nc.scalar.activation(ob[:, oc * _D:(oc + 1) * _D],
                                         xt[t0 + j], SIG,
                                         bias=nu0_col[t0 + j])
                    outdone[t0 + j] = True
                emit_ready_stores()
                return
            de = eng[_DELTA_ENG[u]]
            f0 = st.tile([_P, w], f32, tag=f"f0_{u}", name=f"f0_{u}")
            if ty == "a":
                for j in range(w):
                    ob, oc = ocol[t0 + j]
                    nc.scalar.activation(
                        ob[:, oc * _D:(oc + 1) * _D], xt[t0 + j], SIG,
                        bias=nu0_col[t0 + j],
                        accum_out=f0[:, j:j + 1])
            else:  # 'c': per-tile sigmoid, f0 via DVE row-sum
                for j in range(w):
                    ob, oc = ocol[t0 + j]
                    nc.scalar.activation(
                        ob[:, oc * _D:(oc + 1) * _D], xt[t0 + j], SIG,
                        bias=nu0_col[t0 + j])
                ob0, oc0 = ocol[t0]
                obN, ocN = ocol[t0 + w - 1]
                assert ob0 is obN and ocN == oc0 + w - 1, (u, t0, w)
                nc.vector.tensor_reduce(
                    f0[:], ob0[:, oc0 * _D:(oc0 + w) * _D].rearrange(
                        "p (g d) -> p g d", d=_D),
                    AX.X, A.add)

            dlp = st.tile([_P, w], f32, tag=f"dl_{u}", name=f"dl_{u}")
            if _DELTA_ENG[u] == "p":
                # Pool has no scalar_tensor_tensor; two-op form
                fk = st.tile([_P, w], f32, tag=f"fk_{u}", name=f"fk_{u}")
                nc.gpsimd.tensor_scalar(fk[:], f0[:], 1.0, -_KF,
                                        A.mult, A.add)
                nc.gpsimd.tensor_tensor(dlp[:], fk[:], widen(gg_col[t0], w),
                                        A.mult)
            else:
                nc.vector.scalar_tensor_tensor(dlp[:], f0[:], -_KF,
                                               widen(gg_col[t0], w),
                                               A.add, A.mult)
            dlp_u[u] = dlp
            if fx in ("d", "p"):
                c1 = st.tile([_P, w], f32, tag=f"c1_{u}", name=f"c1_{u}")
                de.tensor_scalar(c1[:], dlp[:], -1.0, 1.0, A.mult, A.add)
                c1_u[u] = c1
            else:
                nu2 = st.tile([_P, w], f32, tag=f"nu2_{u}", name=f"nu2_{u}")
                de.tensor_tensor(nu2[:], widen(nu0_col[t0], w), dlp[:],
                                 A.subtract)
                nu2_u[u] = nu2

        def stage2(u):
            ty, w, fx = units[u]
            t0 = unit_t0[u]
            if not _NEWTON:
                return
            if fx == "d":
                # delta = -dlp:  out = (out0*dlp + (1-dlp)) * out0, in place
                for j in range(w):
                    ob, oc = ocol[t0 + j]
                    dst = ob[:, oc * _D:(oc + 1) * _D]
                    acc = st.tile([_P, 1], f32, tag=f"fac_{u}_{j}",
                                  name=f"fac_{u}_{j}")
                    nc.vector.affine_mul_reduce(
                        dst, acc[:], dst, dst,
                        dlp_u[u][:, j:j + 1], c1_u[u][:, j:j + 1])
                    outdone[t0 + j] = True
            elif fx == "p":
                # same fix on Pool as two ops via a scratch tile
                for j in range(w):
                    ob, oc = ocol[t0 + j]
                    dst = ob[:, oc * _D:(oc + 1) * _D]
                    tmp = pp.tile([_P, _D], f32, tag=f"ft_{u}_{j}",
                                  name=f"ft_{u}_{j}")
                    nc.gpsimd.tensor_scalar(tmp[:], dst,
                                            dlp_u[u][:, j:j + 1],
                                            c1_u[u][:, j:j + 1],
                                            A.mult, A.add)
                    nc.gpsimd.tensor_tensor(dst, tmp[:], dst, A.mult)
                    outdone[t0 + j] = True
            elif fx in ("a", "A"):
                for j in range(w):
                    ob, oc = ocol[t0 + j]
                    nc.scalar.activation(ob[:, oc * _D:(oc + 1) * _D],
                                         xt[t0 + j], SIG,
                                         bias=nu2_u[u][:, j:j + 1])
                    outdone[t0 + j] = True
            else:  # 'w'
                pre2 = pp.tile([_P, w * _D], f32, tag=f"pr2_{u}",
                               name=f"pr2_{u}")
                if _OUP[u] == "p":
                    for j in range(w):
                        nc.gpsimd.tensor_scalar_add(
                            pre2[:, j * _D:(j + 1) * _D],
                            xt[t0 + j], nu2_u[u][:, j:j + 1])
                else:
                    nc.vector.tensor_tensor(
                        pre2[:].rearrange("p (g d) -> p g d", d=_D),
                        xwide(t0, w), bcast(nu2_u[u][:], w), A.add)
                ob0, oc0 = ocol[t0]
                obN, ocN = ocol[t0 + w - 1]
                if ob0 is obN and ocN == oc0 + w - 1:
                    nc.scalar.activation(
                        ob0[:, oc0 * _D:(oc0 + w) * _D], pre2[:], SIG)
                else:
                    for j in range(w):
                        ob, oc = ocol[t0 + j]
                        nc.scalar.activation(
                            ob[:, oc * _D:(oc + 1) * _D],
                            pre2[:, j * _D:(j + 1) * _D], SIG)
                for j in range(w):
                    outdone[t0 + j] = True
            emit_ready_stores()

        # ---- pipelined emission ----
        next_sb = 0
        next_pb = 0

        def stats_upto(tile_limit):
            nonlocal next_sb
            while (next_sb < len(pbatches)
                   and pbatches[next_sb][0] < tile_limit):
                emit_stats(next_sb)
                next_sb += 1

        def polys_upto(tile_limit):
            nonlocal next_pb
            while (next_pb < len(pbatches)
                   and pbatches[next_pb][0] < tile_limit):
                stats_upto(pbatches[next_pb][0] + 1)
                emit_polys(next_pb)
                next_pb += 1

        deferred = []

        def emit_stage2_of(u):
            if u < _LAG:
                return
            v = u - _LAG
            if units[v][2] == "A":
                deferred.append(v)
            else:
                stage2(v)

        for u in range(NU):
            for ch in _EMIT:
                if ch == "p":
                    polys_upto(min(_T, unit_t0[u] + units[u][1]))
                elif ch == "s":
                    stats_upto(min(_T, unit_t0[u] + units[u][1] + _SLA))
                elif ch == "1":
                    stage1(u)
                elif ch == "2":
                    emit_stage2_of(u)
        for v in deferred:
            stage2(v)
        for u in range(max(0, NU - _LAG), NU):
            if units[u][2] == "A":
                stage2(u)
            else:
                stage2(u)
        assert not oblk, oblk

    nc.compile()
    return nc


def _get_nc():
    if "nc" not in _cache:
        _cache["nc"] = _build_nc()
    return _cache["nc"]


def kernel(x: np.ndarray) -> np.ndarray:
    from concourse.bass_utils import run_bass_kernel_spmd

    x = np.ascontiguousarray(x, dtype=np.float32)
    assert x.shape == (_B, _D), x.shape

    nc = _get_nc()
    in_maps = [{"x": x[i * _BC:(i + 1) * _BC]} for i in range(_CORES)]
    res = run_bass_kernel_spmd(nc, in_maps, list(range(_CORES)))
    out = np.concatenate([res.results[i]["y"] for i in range(_CORES)], axis=0)
    return out.astype(np.float32)
